# revision 5
# baseline (speedup 1.0000x reference)
"""AlignNet Trainium2 kernel: 6x conv3x3+BN+ReLU, 3 maxpools, 2 FC, heatmaps.

Self-contained: hardcodes shapes for nn_AlignNet_24120536334875.
Sharding: pure data parallelism, batch 128 -> 16 images per core x 8 cores.

Conv strategy: channels on partitions, 3x3 conv = 9 shifted-AP matmuls
accumulating in PSUM (zero-padded ring layout).  BN+ReLU fused into the
ScalarE epilogue (scale/bias per partition).  FC1 computed transposed
(hT = fw1 @ feat.T) so FC2 and the heatmap parameter gather stay on
partitions.  Heatmaps: separable row/col form m = relu(max_k R_k(h)-C_k(w))
computed with 0-stride broadcast APs on DVE/GPSIMD.
"""
import numpy as np

import concourse.bass as bass
import concourse.bacc as bacc
import concourse.tile as tile
import concourse.mybir as mybir
from concourse.bass_utils import run_bass_kernel_spmd

F32 = mybir.dt.float32
BF16 = mybir.dt.bfloat16
AF = mybir.ActivationFunctionType
OP = mybir.AluOpType

# --- model constants (hardcoded from the problem spec) ---
CROP, MAP, MS, AU, LAND = 176, 44, 52, 15, 49
SR, FC_, BN_EPS = 0.14, 0.56, 1e-5
I1 = np.array([4, 1, 2, 24, 21, 15, 43, 31, 31, 31, 39, 34, 34, 34, 39])
I2 = np.array([5, 8, 7, 29, 26, 17, 45, 37, 37, 37, 41, 40, 40, 40, 41])
OFFV = np.array([-0.5, -1.0 / 3, 1.0 / 3, 1.0, 0.0, -0.5, 0.0, 0.0, 0.0, 0.0,
                 0.5, 0.0, 0.0, 0.0, 0.5], dtype=np.float32)
SCALE = np.float32(float(MS) / CROP)
HALF = 4.0
CVAL = np.float32(FC_ / (MS * SR))
MAGIC = float(3 * 2.0 ** 22)   # 1.5*2^23: round-to-nearest-even via fp32 add
BIG = 1.0e4

CH = [(96, 64), (96, 96), (128, 96), (128, 128), (160, 128), (160, 160)]
N_CORES = 8


def _ceil(a, b):
    return -(-a // b)


def build_nc(n_img=16, prec="f32"):
    """Build the per-core Bass program. Returns (nc, meta)."""
    nc = bacc.Bacc("TRN2", target_bir_lowering=False)
    dt_act = F32 if prec == "f32" else BF16

    # ---------------- DRAM I/O ----------------
    xin_d = nc.dram_tensor("xpad", [n_img * 64, 46 * 46], F32, kind="ExternalInput")
    wd = {}
    # conv weight block tensors: (name, rows, cols)
    wspec = {
        1: [("w1", 64, 9 * 96)],
        2: [("w2", 96, 9 * 96)],
        3: [("w3", 96, 9 * 128)],
        4: [("w4", 128, 9 * 128)],
        5: [("w5", 128, 9 * 160)],
        6: [("w6a", 128, 9 * 160), ("w6b", 32, 9 * 160)],
    }
    for lst in wspec.values():
        for nm, r, c in lst:
            wd[nm] = nc.dram_tensor(nm, [r, c], F32, kind="ExternalInput")
    std = {}
    for li, (co, ci) in enumerate(CH, 1):
        blocks = [(0, min(co, 128))] + ([(128, co - 128)] if co > 128 else [])
        for bi, (m0, msz) in enumerate(blocks):
            nm = f"st{li}" + ("ab"[bi] if len(blocks) > 1 else "")
            std[(li, bi)] = nc.dram_tensor(nm, [msz, 2], F32, kind="ExternalInput")
    fw1a_d = nc.dram_tensor("fw1a", [128, 25 * 512], F32, kind="ExternalInput")
    fw1b_d = nc.dram_tensor("fw1b", [32, 25 * 512], F32, kind="ExternalInput")
    fw2t_d = nc.dram_tensor("fw2t", [512, 98], F32, kind="ExternalInput")
    fb1_d = nc.dram_tensor("fb1", [1, 512], F32, kind="ExternalInput")
    fb2_d = nc.dram_tensor("fb2", [1, 98], F32, kind="ExternalInput")
    sel_d = nc.dram_tensor("selm", [98, 61], F32, kind="ExternalInput")
    off_d = nc.dram_tensor("offm", [16, 30], F32, kind="ExternalInput")
    hc_d = nc.dram_tensor("hcg", [128, 52], F32, kind="ExternalInput")
    idn_d = nc.dram_tensor("idn", [98, 98], F32, kind="ExternalInput")
    ones_d = nc.dram_tensor("onesr", [1, 512], F32, kind="ExternalInput")

    feat_o = nc.dram_tensor("feat_o", [n_img, 4000], F32, kind="ExternalOutput")
    land_o = nc.dram_tensor("land_o", [n_img, 98], F32, kind="ExternalOutput")
    aus_o = nc.dram_tensor("aus_o", [n_img * 15, 2704], F32, kind="ExternalOutput")

    # layer geometry: (Hin(unpadded), pad Hp, chunks rows, pool_after)
    geo = {
        1: dict(W=44, Wp=46, chunks=[(0, 11), (11, 11), (22, 11), (33, 11)]),
        2: dict(W=44, Wp=46, chunks=[(0, 11), (11, 11), (22, 11), (33, 11)]),
        3: dict(W=22, Wp=24, chunks=[(0, 22)]),
        4: dict(W=22, Wp=24, chunks=[(0, 22)]),
        5: dict(W=11, Wp=13, chunks=[(0, 11)]),
        6: dict(W=11, Wp=13, chunks=[(0, 11)]),
    }

    with tile.TileContext(nc) as tc:
        with tc.tile_pool(name="const", bufs=1) as cp, \
             tc.tile_pool(name="fwstream", bufs=3) as fwp, \
             tc.tile_pool(name="pools", bufs=2) as poolp, \
             tc.tile_pool(name="heat", bufs=2) as heatp:
            pp = pfc = None

            # ---- resident constants ----
            wt = {nm: cp.tile([r, c], F32, tag=nm, name=nm)
                  for lst in wspec.values() for nm, r, c in lst}
            for nm in wt:
                nc.sync.dma_start(wt[nm][:], wd[nm][:])
            stt = {}
            for key, d in std.items():
                stt[key] = cp.tile(list(d.shape), F32, tag=f"stt{key}",
                                   name=f"stt{key[0]}_{key[1]}")
                nc.sync.dma_start(stt[key][:], d[:])
            fw2t_t = cp.tile([128, 4 * 98], F32, tag="fw2t")
            for k in range(4):
                nc.sync.dma_start(fw2t_t[:, k * 98:(k + 1) * 98],
                                  fw2t_d[k * 128:(k + 1) * 128, :])
            fb1_t = cp.tile([1, 512], F32, tag="fb1")
            nc.sync.dma_start(fb1_t[:], fb1_d[:])
            fb2_t = cp.tile([1, 98], F32, tag="fb2")
            nc.sync.dma_start(fb2_t[:], fb2_d[:])
            sel_t = cp.tile([98, 61], F32, tag="sel")
            nc.sync.dma_start(sel_t[:], sel_d[:])
            off_t = cp.tile([16, 30], F32, tag="off")
            nc.sync.dma_start(off_t[:], off_d[:])
            hc_t = cp.tile([128, 52], F32, tag="hc")
            nc.sync.dma_start(hc_t[:], hc_d[:])
            idn_t = cp.tile([98, 98], F32, tag="idn")
            nc.sync.dma_start(idn_t[:], idn_d[:])
            ones_t = cp.tile([1, 512], F32, tag="ones")
            nc.sync.dma_start(ones_t[:], ones_d[:])

            # ---- static activation buffers (double buffered, padded ring) ----
            def act_buf(tag, p, free):
                t = [cp.tile([p, free], F32, tag=f"{tag}{i}", name=f"{tag}{i}")
                     for i in range(2)]
                for x in t:
                    nc.vector.memset(x[:], 0.0)
                return t

            xin_b = act_buf("xin", 64, 46 * 46)
            y1_b = act_buf("y1", 96, 46 * 46)
            z2_b = act_buf("z2", 96, 44 * 44)
            y2_b = act_buf("y2", 96, 24 * 24)
            y3_b = act_buf("y3", 128, 24 * 24)
            z4_b = act_buf("z4", 128, 22 * 22)
            y4_b = act_buf("y4", 128, 13 * 13)
            y5a_b = act_buf("y5a", 128, 13 * 13)
            y5b_b = act_buf("y5b", 32, 13 * 13)
            y6a_b = act_buf("y6a", 128, 121)
            y6b_b = act_buf("y6b", 32, 121)
            feat_a = cp.tile([128, n_img * 25], F32, tag="feata")
            feat_b = cp.tile([32, n_img * 25], F32, tag="featb")

            # conv layer plans: per layer, list of K-blocks:
            #   (input_tiles_fn(i) -> tile, p0, psz, wtensor, wrow0)
            def in_tiles(li, i):
                return {1: [xin_b[i % 2]], 2: [y1_b[i % 2]], 3: [y2_b[i % 2]],
                        4: [y3_b[i % 2]], 5: [y4_b[i % 2]],
                        6: [y5a_b[i % 2], y5b_b[i % 2]]}[li]

            def out_tiles(li, i):
                return {1: [y1_b[i % 2]], 2: [z2_b[i % 2]], 3: [y3_b[i % 2]],
                        4: [z4_b[i % 2]], 5: [y5a_b[i % 2], y5b_b[i % 2]],
                        6: [y6a_b[i % 2], y6b_b[i % 2]]}[li]

            # unpadded output layers (raw conv result, consumed by pool only)
            UNPADDED_OUT = {2: 44, 4: 22, 6: 11}

            wnames = {1: ["w1"], 2: ["w2"], 3: ["w3"], 4: ["w4"], 5: ["w5"],
                      6: ["w6a", "w6b"]}

            def conv_layer(li, img):
                co, ci = CH[li - 1]
                g = geo[li]
                W, Wp = g["W"], g["Wp"]
                mblocks = [(0, min(co, 128))] + ([(128, co - 128)] if co > 128 else [])
                xt = in_tiles(li, img)
                ot = out_tiles(li, img)
                pool_after = li in (2, 4)
                for (r0, nr) in g["chunks"]:
                    N = nr * W
                    for mi, (m0, msz) in enumerate(mblocks):
                        ps = pp.tile([128, N], F32, tag="cps", name="cps")[0:msz, :]
                        nmm = 9 * len(xt)
                        k = 0
                        for t9, (dy, dx) in enumerate([(a, b) for a in range(3)
                                                       for b in range(3)]):
                            for xi, xtile in enumerate(xt):
                                wn = wnames[li][xi]
                                lhsT = wt[wn][:, t9 * co + m0: t9 * co + m0 + msz]
                                x3 = xtile[:].rearrange("c (h w) -> c h w", h=Wp)
                                rhs = x3[:, r0 + dy:r0 + dy + nr, dx:dx + W]
                                nc.tensor.matmul(ps[:], lhsT, rhs,
                                                 start=(k == 0), stop=(k == nmm - 1))
                                k += 1
                        # epilogue: relu(ps*s + t') -> write out tile
                        st = stt[(li, mi)]
                        if li in UNPADDED_OUT:
                            dst = ot[mi][:].rearrange("c (h w) -> c h w", h=W)
                            dview = dst[:, r0:r0 + nr, 0:W]
                        else:
                            op_ = ot[mi][:].rearrange("c (h w) -> c h w", h=Wp)
                            dview = op_[:, 1 + r0:1 + r0 + nr, 1:1 + W]
                        nc.scalar.activation(
                            dview, ps[:].rearrange("c (h w) -> c h w", h=nr),
                            AF.Relu, bias=st[:, 1:2], scale=st[:, 0:1])

            def pool22(src, dst, W, Wp2):
                # src raw [c, W, W] -> dst padded interior [c, W/2, W/2]
                c = src.shape[0]
                Wh = W // 2
                si = src[:].rearrange("c (h w) -> c h w", h=W)
                tmp = poolp.tile([c, W * Wh], F32, tag=f"ptmp{c}x{W}", name=f"ptmp{c}x{W}")
                t3 = tmp[:].rearrange("c (h w) -> c h w", h=W)
                nc.vector.tensor_tensor(t3[:, :, :], si[:, :, 0:2 * Wh:2],
                                        si[:, :, 1:2 * Wh:2], op=OP.max)
                d3 = dst[:].rearrange("c (h w) -> c h w", h=Wp2)
                dv = d3[:, 1:1 + Wh, 1:1 + Wh]
                nc.vector.tensor_tensor(dv, t3[:, 0:2 * Wh:2, :],
                                        t3[:, 1:2 * Wh:2, :], op=OP.max)

            def pool_final(img):
                # y6 [c,11,11] -> feat [c, img*25 : +25] (VALID 11->5)
                for src, ft in ((y6a_b[img % 2], feat_a), (y6b_b[img % 2], feat_b)):
                    c = src.shape[0]
                    s3 = src[:].rearrange("c (h w) -> c h w", h=11)
                    tmp = poolp.tile([c, 11 * 5], F32, tag=f"pf{c}", name=f"pf{c}")
                    t3 = tmp[:].rearrange("c (h w) -> c h w", h=11)
                    nc.vector.tensor_tensor(t3[:, :, :], s3[:, :, 0:10:2],
                                            s3[:, :, 1:10:2], op=OP.max)
                    dv = ft[:, img * 25:(img + 1) * 25].rearrange(
                        "c (h w) -> c h w", h=5)
                    nc.vector.tensor_tensor(dv, t3[:, 0:10:2, :], t3[:, 1:10:2, :],
                                            op=OP.max)

            # ================= conv stack =================
            conv_psum_ctx = tc.tile_pool(name="psum", bufs=4, space="PSUM")
            pp = conv_psum_ctx.__enter__()
            for img in range(n_img):
                nc.sync.dma_start(xin_b[img % 2][:],
                                  xin_d[img * 64:(img + 1) * 64, :])
                conv_layer(1, img)
                conv_layer(2, img)
                pool22(z2_b[img % 2], y2_b[img % 2], 44, 24)
                conv_layer(3, img)
                conv_layer(4, img)
                pool22(z4_b[img % 2], y4_b[img % 2], 22, 13)
                conv_layer(5, img)
                conv_layer(6, img)
                pool_final(img)

            conv_psum_ctx.__exit__(None, None, None)
            fc_psum_ctx = tc.tile_pool(name="psfc", bufs=1, space="PSUM")
            pfc = fc_psum_ctx.__enter__()
            # ================= FC1 (transposed): hT [512,16] =================
            hT_ps = [pfc.tile([128, n_img], F32, tag=f"hps{m}", name=f"hps{m}")
                     for m in range(4)]
            for s in range(25):
                fa = fwp.tile([128, 512], F32, tag="fw1a", name="fw1achunk")
                nc.sync.dma_start(fa[:], fw1a_d[:, s * 512:(s + 1) * 512])
                fb = fwp.tile([32, 512], F32, tag="fw1b", name="fw1bchunk")
                nc.sync.dma_start(fb[:], fw1b_d[:, s * 512:(s + 1) * 512])
                for m in range(4):
                    ra = feat_a[:, s::25]
                    nc.tensor.matmul(hT_ps[m][:], fa[:, m * 128:(m + 1) * 128], ra,
                                     start=(s == 0), stop=False)
                    rb = feat_b[:, s::25]
                    nc.tensor.matmul(hT_ps[m][:], fb[:, m * 128:(m + 1) * 128], rb,
                                     start=False, stop=False)
            for m in range(4):
                nc.tensor.matmul(hT_ps[m][:], fb1_t[0:1, m * 128:(m + 1) * 128],
                                 ones_t[0:1, 0:n_img], start=False, stop=True)
            hTs = cp.tile([128, 4 * n_img], F32, tag="hTs")
            for m in range(4):
                nc.scalar.copy(hTs[:, m * n_img:(m + 1) * n_img], hT_ps[m][:])

            # ================= FC2: landT [98,16] =================
            lt_ps = pfc.tile([98, n_img], F32, tag="ltps")
            for k in range(4):
                nc.tensor.matmul(lt_ps[:], fw2t_t[:, k * 98:(k + 1) * 98],
                                 hTs[:, k * n_img:(k + 1) * n_img],
                                 start=(k == 0), stop=False)
            nc.tensor.matmul(lt_ps[:], fb2_t[0:1, :], ones_t[0:1, 0:n_img],
                             start=False, stop=True)
            landT = cp.tile([98, n_img], F32, tag="landT")
            nc.scalar.copy(landT[:], lt_ps[:])

            # land output: transpose -> [16, 98] -> DMA
            land_ps = pfc.tile([n_img, 98], F32, tag="landps")
            nc.tensor.transpose(land_ps[:], landT[:], idn_t[:])
            land_sb = cp.tile([n_img, 98], F32, tag="landsb")
            nc.scalar.copy(land_sb[:], land_ps[:])
            nc.sync.dma_start(land_o[:], land_sb[:])

            # feat output
            fav = feat_a[:].rearrange("c (b s) -> c b s", s=25)
            dav = feat_o[:].rearrange("b (c s) -> c b s", c=160)
            nc.sync.dma_start(dav[0:128], fav)
            fbv = feat_b[:].rearrange("c (b s) -> c b s", s=25)
            nc.sync.dma_start(dav[128:160], fbv)

            # ================= heatmap params =================
            # GT [16, 61] = landT.T @ Sel ; col0 = xs22-xs25, 1:31 = cx1|cx2,
            # 31:61 = cy1|cy2
            gt_ps = pfc.tile([n_img, 61], F32, tag="gtps")
            nc.tensor.matmul(gt_ps[:], landT[:], sel_t[:], start=True, stop=True)
            gts = cp.tile([n_img, 61], F32, tag="gts")
            nc.scalar.copy(gts[:], gt_ps[:])
            ruler = cp.tile([n_img, 1], F32, tag="ruler")
            nc.scalar.activation(ruler[:], gts[:, 0:1], AF.Abs)
            ay_pre = cp.tile([n_img, 30], F32, tag="aypre")
            nc.vector.scalar_tensor_tensor(ay_pre[:], off_t[0:n_img, :],
                                           ruler[:, 0:1], gts[:, 31:61],
                                           op0=OP.mult, op1=OP.add)
            # V [16, 15*12] au-major params:
            # j: 0 ax1 1 ax2 2 ay1 3 ay2 4 sw1 5 sw2 6 sh1 7 sh2 8 ew1 9 ew2
            #    10 eh1 11 eh2
            V = cp.tile([n_img, 180], F32, tag="V")
            tmp1 = cp.tile([n_img, 60], F32, tag="tmp1")
            # scale mult (ax from gts cols 1:31, ay from ay_pre)
            nc.vector.tensor_scalar_mul(tmp1[:, 0:30], gts[:, 1:31], float(SCALE))
            nc.vector.tensor_scalar_mul(tmp1[:, 30:60], ay_pre[:], float(SCALE))
            nc.vector.tensor_scalar_add(tmp1[:], tmp1[:], MAGIC)
            # write rounded into V axy slots (j,au) iter matches (group, au)
            vv = V[:].rearrange("b (au j) -> b j au", j=12)
            nc.vector.tensor_scalar_add(vv[:, 0:4, :], tmp1[:].rearrange(
                "b (j au) -> b j au", j=4), -MAGIC)
            # S = clip(axy-4, 0, 51), E = clip(axy+4, 0, 51)
            nc.vector.tensor_scalar(vv[:, 4:8, :], vv[:, 0:4, :], -HALF, 0.0,
                                    op0=OP.add, op1=OP.max)
            nc.vector.tensor_scalar_min(vv[:, 4:8, :], vv[:, 4:8, :], 51.0)
            nc.vector.tensor_scalar(vv[:, 8:12, :], vv[:, 0:4, :], HALF, 0.0,
                                    op0=OP.add, op1=OP.max)
            nc.vector.tensor_scalar_min(vv[:, 8:12, :], vv[:, 8:12, :], 51.0)

            # scatter V -> PB blocks [nb*15, 12]
            nblk = _ceil(n_img, 8)
            PB = []
            for blk in range(nblk):
                nb = min(8, n_img - blk * 8)
                pb = cp.tile([nb * 15, 12], F32, tag=f"PB{blk}", name=f"PB{blk}")
                for b8 in range(nb):
                    nc.sync.dma_start(pb[b8 * 15:(b8 + 1) * 15, :],
                                      V[blk * 8 + b8:blk * 8 + b8 + 1, :])
                PB.append(pb)

            # R/C vectors + outer combine per block
            for blk in range(nblk):
                nb = min(8, n_img - blk * 8)
                P = nb * 15
                pb = PB[blk]
                eng = nc.vector
                RC = cp.tile([P, 4 * 52], F32, tag=f"RC{blk}", name=f"RC{blk}")
                U = cp.tile([P, 52], F32, tag=f"U{blk}", name=f"U{blk}")
                hc = hc_t[0:P, :]
                for k in range(2):
                    rsl = RC[:, k * 52:(k + 1) * 52]
                    csl = RC[:, (2 + k) * 52:(3 + k) * 52]
                    # R_k = 1 - c*|h-ay| - BIG*(h<sh) - BIG*(h>eh)
                    nc.vector.tensor_scalar(U[:], hc, pb[:, 2 + k:3 + k],
                                            None, op0=OP.subtract)
                    nc.vector.scalar_tensor_tensor(rsl, U[:], -1.0, U[:],
                                                   op0=OP.mult, op1=OP.max)
                    nc.vector.tensor_scalar(rsl, rsl, float(-CVAL), 1.0,
                                            op0=OP.mult, op1=OP.add)
                    nc.vector.tensor_scalar(U[:], hc, pb[:, 6 + k:7 + k],
                                            None, op0=OP.is_lt)
                    nc.vector.scalar_tensor_tensor(rsl, U[:], -BIG, rsl,
                                                   op0=OP.mult, op1=OP.add)
                    nc.vector.tensor_scalar(U[:], hc, pb[:, 10 + k:11 + k],
                                            None, op0=OP.is_gt)
                    nc.vector.scalar_tensor_tensor(rsl, U[:], -BIG, rsl,
                                                   op0=OP.mult, op1=OP.add)
                    # C_k = c*|w-ax| + BIG*(w<sw) + BIG*(w>ew)
                    nc.vector.tensor_scalar(U[:], hc, pb[:, 0 + k:1 + k],
                                            None, op0=OP.subtract)
                    nc.vector.scalar_tensor_tensor(csl, U[:], -1.0, U[:],
                                                   op0=OP.mult, op1=OP.max)
                    nc.vector.tensor_scalar_mul(csl, csl, float(CVAL))
                    nc.vector.tensor_scalar(U[:], hc, pb[:, 4 + k:5 + k],
                                            None, op0=OP.is_lt)
                    nc.vector.scalar_tensor_tensor(csl, U[:], BIG, csl,
                                                   op0=OP.mult, op1=OP.add)
                    nc.vector.tensor_scalar(U[:], hc, pb[:, 8 + k:9 + k],
                                            None, op0=OP.is_gt)
                    nc.vector.scalar_tensor_tensor(csl, U[:], BIG, csl,
                                                   op0=OP.mult, op1=OP.add)

                def bc(sl, hdim, h0, hn):
                    a = sl
                    if hdim:   # broadcast over w: [p, hn, 52] reading [p, hn]
                        ap = [[a.ap[0][0], P], [1, hn], [0, 52]]
                        off = a.offset + h0
                    else:      # broadcast over h: [p, hn, 52] reading [p, 52]
                        ap = [[a.ap[0][0], P], [0, hn], [1, 52]]
                        off = a.offset
                    return bass.AP(tensor=a.tensor, offset=off, ap=ap)

                # outer combine in 4 h-chunks of 13
                for hc_i in range(4):
                    h0, hn = hc_i * 13, 13
                    m1 = heatp.tile([P, hn * 52], F32, tag=f"m1_{blk}", name=f"m1_{blk}")
                    m2 = heatp.tile([P, hn * 52], F32, tag=f"m2_{blk}", name=f"m2_{blk}")
                    r1 = RC[:, 0:52]; r2 = RC[:, 52:104]
                    c1 = RC[:, 104:156]; c2 = RC[:, 156:208]
                    v1 = m1[:].rearrange("p (h w) -> p h w", h=hn)
                    v2 = m2[:].rearrange("p (h w) -> p h w", h=hn)
                    eng.tensor_tensor(v1, bc(r1, True, h0, hn),
                                      bc(c1, False, h0, hn), op=OP.subtract)
                    eng.tensor_tensor(v2, bc(r2, True, h0, hn),
                                      bc(c2, False, h0, hn), op=OP.subtract)
                    eng.tensor_tensor(m1[:], m1[:], m2[:], op=OP.max)
                    nc.scalar.activation(m1[:], m1[:], AF.Relu)
                    nc.sync.dma_start(
                        aus_o[blk * 120:blk * 120 + P,
                              h0 * 52:(h0 + hn) * 52], m1[:])
            fc_psum_ctx.__exit__(None, None, None)

    nc.compile()
    return nc


# ---------------- host-side prep ----------------
def prep_inputs(inputs, n_img=16, core=0):
    B0 = core * n_img
    x = inputs["x"][B0:B0 + n_img]          # [n,64,44,44]
    xp = np.zeros((n_img, 64, 46, 46), np.float32)
    xp[:, :, 1:45, 1:45] = x
    d = {"xpad": xp.reshape(n_img * 64, 46 * 46)}
    return d


def prep_shared(inputs):
    d = {}
    for li, (co, ci) in enumerate(CH, 1):
        w = inputs[f"w{li}"]                 # [co, ci, 3, 3]
        wtap = w.transpose(2, 3, 1, 0).reshape(9, ci, co)   # (dy,dx),ci,co
        wm = wtap.transpose(1, 0, 2).reshape(ci, 9 * co)    # [ci, 9*co]
        if li == 6:
            d["w6a"] = np.ascontiguousarray(wm[:128])
            d["w6b"] = np.ascontiguousarray(wm[128:])
        else:
            d[f"w{li}"] = np.ascontiguousarray(wm)
        s = inputs[f"s{li}"].astype(np.float32)
        tp = inputs[f"b{li}"].astype(np.float32) * s + inputs[f"t{li}"].astype(np.float32)
        stm = np.stack([s, tp], 1)           # [co, 2]
        if co > 128:
            d[f"st{li}a"] = np.ascontiguousarray(stm[:128])
            d[f"st{li}b"] = np.ascontiguousarray(stm[128:])
        else:
            d[f"st{li}"] = np.ascontiguousarray(stm)
    fw1 = inputs["fw1"]                      # [512, 4000]
    fw1_rs = fw1.reshape(512, 160, 25).transpose(1, 2, 0)   # [c, s, j]
    fw1_rs = np.ascontiguousarray(fw1_rs.reshape(160, 25 * 512)).astype(np.float32)
    d["fw1a"] = np.ascontiguousarray(fw1_rs[:128])
    d["fw1b"] = np.ascontiguousarray(fw1_rs[128:])
    d["fw2t"] = np.ascontiguousarray(inputs["fw2"].T.astype(np.float32))  # [512,98]
    d["fb1"] = inputs["fb1"].reshape(1, 512).astype(np.float32)
    d["fb2"] = inputs["fb2"].reshape(1, 98).astype(np.float32)
    sel = np.zeros((98, 61), np.float32)
    sel[44, 0] = 1.0
    sel[50, 0] = -1.0
    for j in range(15):
        sel[2 * I1[j], 1 + j] = 1.0
        sel[2 * I2[j], 16 + j] = 1.0
        sel[2 * I1[j] + 1, 31 + j] = 1.0
        sel[2 * I2[j] + 1, 46 + j] = 1.0
    d["selm"] = sel
    d["offm"] = np.tile(np.concatenate([OFFV, OFFV])[None, :], (16, 1)).astype(np.float32)
    d["hcg"] = np.tile(np.arange(52, dtype=np.float32)[None, :], (128, 1))
    d["idn"] = np.eye(98, dtype=np.float32)
    d["onesr"] = np.ones((1, 512), np.float32)
    return d


_CACHED = {}


def _get_nc(n_img=16, prec="f32"):
    key = (n_img, prec)
    if key not in _CACHED:
        _CACHED[key] = build_nc(n_img, prec)
    return _CACHED[key]


def kernel(**inputs):
    n_img = 128 // N_CORES
    nc = _get_nc(n_img)
    shared = prep_shared(inputs)
    in_maps = []
    for c in range(N_CORES):
        m = dict(shared)
        m.update(prep_inputs(inputs, n_img, c))
        in_maps.append(m)
    res = run_bass_kernel_spmd(nc, in_maps, core_ids=list(range(N_CORES)))
    feat = np.concatenate([r["feat_o"] for r in res.results], 0)
    land = np.concatenate([r["land_o"] for r in res.results], 0)
    aus = np.concatenate([r["aus_o"] for r in res.results], 0)
    return (feat.reshape(128, 160, 5, 5), land.reshape(128, 98),
            aus.reshape(128, 15, 52, 52))
